# revision 3
# baseline (speedup 1.0000x reference)
"""Trainium2 Bass kernel for nn_EqvSelfAttention (B=4, N=1024, D=256, H=8).

Sharding: data-parallel over (batch b, query-half) -> 8 cores.
Each core computes all 8 heads for its 512 query rows against all 1024 keys.

Math notes (vs reference):
  * 1/sqrt(D)=1/16 folded into Wq (exact power of two).
  * Per-head location-bias MLP: loc_h = sum_d wg2[h,d]*relu(hid_hd) + bg2[h].
    - |wg2| folded into layer-1 weights/bias => z'_hd; sign applied in the
      PE "reduce" matmul that accumulates loc directly onto the content
      logits in PSUM (transposed layout [key, query]).
    - bg2 dropped: constant across keys => softmax-invariant.
  * Softmax computed without max subtraction (logits are O(+-6), exp is safe
    in fp32). Key presence mask folded into V'' = [pk*V | pk]; the 33rd
    column of the A@V'' matmul yields the softmax denominator Z.
  * Absent queries (pq=0) produce uniform attention over ALL keys in the
    reference => Oh = mean(V). Handled by blending with mean(V) after.
"""

import sys
import numpy as np

sys.path.insert(0, "/opt/trn_rl_repo")

B, N, D, H, DH = 4, 1024, 256, 8, 32
R = 512  # query rows per core
NCORES = 8

_CACHE = {}


def _build_program():
    from contextlib import ExitStack

    from concourse import bass, mybir
    import concourse.tile as tile
    from concourse.masks import make_identity

    f32 = mybir.dt.float32
    AF = mybir.ActivationFunctionType
    OP = mybir.AluOpType
    ds = bass.ds

    nc = bass.Bass("TRN2", target_bir_lowering=False, debug=False)

    # ---- I/O declarations (order matters for the PJRT call) ----
    d_y = nc.declare_dram_parameter("y", [N, D], f32, isOutput=False)
    d_yq = nc.declare_dram_parameter("yq", [R, D], f32, isOutput=False)
    d_xp = nc.declare_dram_parameter("xp", [R, 3 * N], f32, isOutput=False)
    d_pkc = nc.declare_dram_parameter("pkc", [128, 8], f32, isOutput=False)
    d_pqr = nc.declare_dram_parameter("pqr", [1, R], f32, isOutput=False)
    d_pqcr = nc.declare_dram_parameter("pqcr", [1, R], f32, isOutput=False)
    d_wq = nc.declare_dram_parameter("wq", [D, D], f32, isOutput=False)
    d_wk = nc.declare_dram_parameter("wk", [D, D], f32, isOutput=False)
    d_wv = nc.declare_dram_parameter("wv", [D, D], f32, isOutput=False)
    d_wo = nc.declare_dram_parameter("wo", [D, D], f32, isOutput=False)
    d_bq = nc.declare_dram_parameter("bq", [1, D], f32, isOutput=False)
    d_bk = nc.declare_dram_parameter("bk", [1, D], f32, isOutput=False)
    d_bv = nc.declare_dram_parameter("bv", [1, D], f32, isOutput=False)
    d_bo = nc.declare_dram_parameter("bo", [1, D], f32, isOutput=False)
    d_bd = nc.declare_dram_parameter("bd", [H, 96, 128], f32, isOutput=False)
    d_rb = nc.declare_dram_parameter("rb", [128, H], f32, isOutput=False)
    d_lr = nc.declare_dram_parameter("lr", [H, 4, 128, 128], f32, isOutput=False)
    d_o = nc.declare_dram_parameter("o", [R, D], f32, isOutput=True)

    with tile.TileContext(nc) as tc:
        with ExitStack() as ctx:
            consts = ctx.enter_context(tc.tile_pool(name="consts", bufs=1))
            persist = ctx.enter_context(tc.tile_pool(name="persist", bufs=1))

            # ---------- constants ----------
            ident = consts.tile([128, 128], f32)
            make_identity(nc, ident)
            ones512 = consts.tile([1, 512], f32)
            nc.vector.memset(ones512, 1.0)
            ones128r = consts.tile([1, 128], f32)
            nc.vector.memset(ones128r, 1.0)
            inv1024c = consts.tile([128, 1], f32)
            nc.vector.memset(inv1024c, 1.0 / 1024.0)

            wqs = consts.tile([128, 2, D], f32)
            nc.sync.dma_start(wqs, d_wq[:, :].rearrange("(t p) d -> p t d", p=128))
            wks = consts.tile([128, 2, D], f32)
            nc.sync.dma_start(wks, d_wk[:, :].rearrange("(t p) d -> p t d", p=128))
            wvs = consts.tile([128, 2, D], f32)
            nc.sync.dma_start(wvs, d_wv[:, :].rearrange("(t p) d -> p t d", p=128))
            wos = consts.tile([128, 2, D], f32)
            nc.sync.dma_start(wos, d_wo[:, :].rearrange("(t p) d -> p t d", p=128))
            bqs = consts.tile([1, D], f32)
            nc.sync.dma_start(bqs, d_bq[:, :])
            bks = consts.tile([1, D], f32)
            nc.sync.dma_start(bks, d_bk[:, :])
            bvs = consts.tile([1, D], f32)
            nc.sync.dma_start(bvs, d_bv[:, :])
            bos = consts.tile([1, D], f32)
            nc.sync.dma_start(bos, d_bo[:, :])
            bdsb = consts.tile([96, H, 128], f32)
            nc.sync.dma_start(bdsb, d_bd[:, :, :].rearrange("h p m -> p h m"))
            rbsb = consts.tile([128, H], f32)
            nc.sync.dma_start(rbsb, d_rb[:, :])
            lrsb = consts.tile([128, H, 4, 128], f32)
            nc.sync.dma_start(lrsb, d_lr[:, :, :, :].rearrange("h c p m -> p h c m"))
            pkcs = consts.tile([128, 8], f32)
            nc.sync.dma_start(pkcs, d_pkc[:, :])
            pqs = consts.tile([1, R], f32)
            nc.sync.dma_start(pqs, d_pqr[:, :])
            pqcs = consts.tile([1, R], f32)
            nc.sync.dma_start(pqcs, d_pqcr[:, :])

            # ---------- persistent activations ----------
            ktsb = persist.tile([128, 2, N], f32)     # K^T [dout, key]
            qtz = persist.tile([128, H, 512], f32)    # per-head zero-padded Q^T
            vsb = persist.tile([128, 8, D], f32)      # V [key, dout]
            v2sb = persist.tile([128, 8, H, 33], f32)  # [pk*V_h | pk]
            vtsb = persist.tile([128, 2, R], f32)     # V^T of my rows
            mvt = persist.tile([128, 2], f32)         # mean_k V  (transposed col)
            xtall = persist.tile([128, 8, 4, 512], f32)  # Xp^T (96 rows used)
            otsb = persist.tile([128, 2, R], f32)     # O^T accumulator
            pqcb = persist.tile([128, R], f32)        # (1-pq) replicated rows

            nc.gpsimd.memset(qtz, 0.0)

            # ---------- phase A: Y^T and projections ----------
            with tc.tile_pool(name="ph_a", bufs=1) as pha, \
                 tc.tile_pool(name="ps_a", bufs=2, space="PSUM") as psa:
                ysb = pha.tile([128, 8, D], f32)
                nc.sync.dma_start(ysb, d_y[:, :].rearrange("(t p) d -> p t d", p=128))
                ysq = pha.tile([128, 4, D], f32)
                nc.sync.dma_start(ysq, d_yq[:, :].rearrange("(t p) d -> p t d", p=128))

                yt = pha.tile([128, 2, N], f32)   # Y^T full batch
                ytq = pha.tile([128, 2, R], f32)  # Y^T my rows
                for dt_ in range(2):
                    for g in range(2):  # groups of 4 n-tiles
                        ps = psa.tile([128, 512], f32)
                        for j in range(4):
                            nt = g * 4 + j
                            nc.tensor.transpose(
                                ps[:, ds(128 * j, 128)],
                                ysb[:, nt, ds(128 * dt_, 128)],
                                ident,
                            )
                        nc.vector.tensor_copy(yt[:, dt_, ds(512 * g, 512)], ps)
                    ps = psa.tile([128, 512], f32)
                    for j in range(4):
                        nc.tensor.transpose(
                            ps[:, ds(128 * j, 128)],
                            ysq[:, j, ds(128 * dt_, 128)],
                            ident,
                        )
                    nc.vector.tensor_copy(ytq[:, dt_], ps)

                qtsb = pha.tile([128, 2, R], f32)
                # Q^T (scaled Wq), K^T, V, V^T projections
                for dt_ in range(2):
                    ps = psa.tile([128, 512], f32)
                    for k_ in range(2):
                        nc.tensor.matmul(
                            ps, wqs[:, k_, ds(128 * dt_, 128)], ytq[:, k_],
                            start=(k_ == 0), stop=False,
                        )
                    nc.tensor.matmul(
                        ps, bqs[0:1, ds(128 * dt_, 128)], ones512,
                        start=False, stop=True,
                    )
                    nc.vector.tensor_copy(qtsb[:, dt_], ps)

                    for half in range(2):
                        ps = psa.tile([128, 512], f32)
                        for k_ in range(2):
                            nc.tensor.matmul(
                                ps, wks[:, k_, ds(128 * dt_, 128)],
                                yt[:, k_, ds(512 * half, 512)],
                                start=(k_ == 0), stop=False,
                            )
                        nc.tensor.matmul(
                            ps, bks[0:1, ds(128 * dt_, 128)], ones512,
                            start=False, stop=True,
                        )
                        nc.vector.tensor_copy(ktsb[:, dt_, ds(512 * half, 512)], ps)

                    ps = psa.tile([128, 512], f32)
                    for k_ in range(2):
                        nc.tensor.matmul(
                            ps, wvs[:, k_, ds(128 * dt_, 128)], ytq[:, k_],
                            start=(k_ == 0), stop=False,
                        )
                    nc.tensor.matmul(
                        ps, bvs[0:1, ds(128 * dt_, 128)], ones512,
                        start=False, stop=True,
                    )
                    nc.vector.tensor_copy(vtsb[:, dt_], ps)

                for nt in range(8):
                    ps = psa.tile([128, 256], f32)
                    for k_ in range(2):
                        nc.tensor.matmul(
                            ps, yt[:, k_, ds(128 * nt, 128)], wvs[:, k_],
                            start=(k_ == 0), stop=False,
                        )
                    nc.tensor.matmul(ps, ones128r, bvs, start=False, stop=True)
                    nc.vector.tensor_copy(vsb[:, nt], ps)

                # per-head zero-padded Q^T slices (keeps content matmuls K=128)
                for h in range(H):
                    base = 32 * (h % 4)
                    nc.vector.tensor_copy(
                        qtz[ds(base, 32), h], qtsb[ds(base, 32), h // 4]
                    )

                # V'' = [pk * V_h | pk]
                for nt in range(8):
                    nc.vector.tensor_scalar(
                        v2sb[:, nt, :, 0:32],
                        vsb[:, nt].rearrange("p (h d) -> p h d", h=H),
                        pkcs[:, nt : nt + 1],
                        None,
                        op0=OP.mult,
                    )
                    nc.vector.tensor_copy(
                        v2sb[:, nt, :, 32:33],
                        pkcs[:, nt : nt + 1].to_broadcast((128, H, 1)),
                    )

                # mean_k V (transposed): mvt[d] = sum_n V[n, d] / 1024
                psmv = psa.tile([128, 2], f32)
                for dt_ in range(2):
                    for nt in range(8):
                        nc.tensor.matmul(
                            psmv[:, dt_ : dt_ + 1],
                            vsb[:, nt, ds(128 * dt_, 128)],
                            inv1024c,
                            start=(nt == 0), stop=(nt == 7),
                        )
                nc.vector.tensor_copy(mvt, psmv)

            # ---------- phase B0: transpose X_pairs ----------
            with tc.tile_pool(name="xp_in", bufs=2) as xpin, \
                 tc.tile_pool(name="ps_t", bufs=2, space="PSUM") as pst:
                for kt in range(8):
                    xt_in = xpin.tile([128, 4, 384], f32)
                    for qt in range(4):
                        nc.sync.dma_start(
                            xt_in[:, qt],
                            d_xp[ds(128 * qt, 128), ds(384 * kt, 384)],
                        )
                    for cp in range(2):  # chunk pairs
                        ps = pst.tile([128, 2, 512], f32)
                        for ci in range(2):
                            for qt in range(4):
                                nc.tensor.transpose(
                                    ps[0:96, ci, ds(128 * qt, 128)],
                                    xt_in[:, qt, ds(96 * (2 * cp + ci), 96)],
                                    ident,
                                )
                        if cp % 2 == 0:
                            nc.scalar.copy(
                                xtall[0:96, kt, ds(2 * cp, 2)], ps[0:96]
                            )
                        else:
                            nc.vector.tensor_copy(
                                xtall[0:96, kt, ds(2 * cp, 2)], ps[0:96]
                            )

            # ---------- phase B1: attention main loop ----------
            with tc.tile_pool(name="ps_ct", bufs=2, space="PSUM") as psct, \
                 tc.tile_pool(name="ps_z", bufs=2, space="PSUM") as psz, \
                 tc.tile_pool(name="ps_av", bufs=2, space="PSUM") as psav, \
                 tc.tile_pool(name="rz_p", bufs=2) as rzp, \
                 tc.tile_pool(name="et_p", bufs=2) as etp, \
                 tc.tile_pool(name="fin_p", bufs=2) as finp:
                # replicate (1-pq) across partitions via a K=1 outer product
                psb = psct.tile([128, 512], f32, name="psbc", tag="ct")
                nc.tensor.matmul(psb, ones128r, pqcs, start=True, stop=True)
                nc.vector.tensor_copy(pqcb, psb)
                for h in range(H):
                    av = psav.tile([128, 512], f32)
                    for kt in range(8):
                        ct = psct.tile([128, 512], f32, name="ct", tag="ct")
                        nc.tensor.matmul(
                            ct,
                            ktsb[:, h // 4, ds(128 * kt, 128)],
                            qtz[:, h],
                            start=True, stop=False,
                        )
                        rzs = []
                        for cp in range(2):
                            zps = psz.tile([128, 2, 512], f32)
                            for ci in range(2):
                                nc.tensor.matmul(
                                    zps[:, ci], bdsb[:, h],
                                    xtall[0:96, kt, 2 * cp + ci],
                                    start=True, stop=True,
                                )
                            rz = rzp.tile([128, 2, 512], f32)
                            if cp % 2 == 0:
                                nc.scalar.activation(
                                    rz, zps, AF.Relu, bias=rbsb[:, h : h + 1]
                                )
                            else:
                                nc.vector.tensor_scalar(
                                    rz, zps, rbsb[:, h : h + 1], 0.0,
                                    op0=OP.add, op1=OP.max,
                                )
                            rzs.append(rz)
                        for c4 in range(4):
                            nc.tensor.matmul(
                                ct, lrsb[:, h, c4], rzs[c4 // 2][:, c4 % 2],
                                start=False, stop=(c4 == 3),
                            )
                        et = etp.tile([128, 512], f32)
                        nc.scalar.activation(et, ct, AF.Exp)
                        nc.tensor.matmul(
                            av[0:33], v2sb[:, kt, h], et,
                            start=(kt == 0), stop=(kt == 7),
                        )
                    # finalize head h
                    rec = finp.tile([1, 512], f32)
                    nc.vector.reciprocal(rec, av[32:33])
                    rpq = finp.tile([1, 512], f32)
                    nc.vector.tensor_mul(rpq, rec, pqs)
                    nc.tensor.matmul(
                        av[64:96], ones128r[0:1, 0:32], rpq, start=True, stop=True
                    )
                    rpqs = finp.tile([32, 512], f32)
                    nc.vector.tensor_copy(rpqs, av[64:96])
                    t2 = finp.tile([32, 512], f32)
                    nc.vector.tensor_mul(t2, av[0:32], rpqs)
                    mv0 = finp.tile([32, 1], f32)
                    nc.vector.tensor_copy(
                        mv0, mvt[ds(32 * (h % 4), 32), h // 4 : h // 4 + 1]
                    )
                    t3 = finp.tile([32, 512], f32)
                    nc.vector.tensor_scalar(
                        t3, pqcb[0:32], mv0, None, op0=OP.mult
                    )
                    t4 = finp.tile([32, 512], f32)
                    nc.vector.tensor_add(t4, t2, t3)
                    vt0 = finp.tile([32, 512], f32)
                    nc.vector.tensor_copy(vt0, vtsb[ds(32 * (h % 4), 32), h // 4])
                    nc.vector.tensor_add(
                        otsb[ds(32 * (h % 4), 32), h // 4], t4, vt0
                    )

            # ---------- phase C: O = O + relu(O @ Wo + bo) ----------
            with tc.tile_pool(name="ps_o", bufs=2, space="PSUM") as pso, \
                 tc.tile_pool(name="o_p", bufs=2) as op_:
                for j in range(4):
                    pso1 = pso.tile([128, 256], f32)
                    for dt_ in range(2):
                        nc.tensor.transpose(
                            pso1[:, ds(128 * dt_, 128)],
                            otsb[:, dt_, ds(128 * j, 128)],
                            ident,
                        )
                    oj = op_.tile([128, 256], f32)
                    nc.vector.tensor_copy(oj, pso1)

                    pso2 = pso.tile([128, 256], f32)
                    for dt_ in range(2):
                        nc.tensor.matmul(
                            pso2, otsb[:, dt_, ds(128 * j, 128)], wos[:, dt_],
                            start=(dt_ == 0), stop=False,
                        )
                    nc.tensor.matmul(pso2, ones128r, bos, start=False, stop=True)
                    r2 = op_.tile([128, 256], f32)
                    nc.scalar.activation(r2, pso2, AF.Relu)
                    ofin = op_.tile([128, 256], f32)
                    nc.vector.tensor_add(ofin, oj, r2)
                    nc.sync.dma_start(d_o[ds(128 * j, 128), :], ofin)

    _split_multiwait(nc, mybir)
    return nc


def _split_multiwait(nc, mybir):
    """This walrus build only encodes ONE sem-wait per instruction; Tile's
    tail drain carries several. Split extras onto preceding NoOps."""
    for f in nc.m.functions:
        for blk in f.blocks:
            insts = list(blk.instructions)
            changed = False
            newlist = []
            for ins in insts:
                si = ins.sync_info
                if si is not None and len(si.on_wait) > 1:
                    waits = list(si.on_wait)
                    for j, w in enumerate(waits[:-1]):
                        newlist.append(
                            mybir.InstNoOp(
                                name=f"{ins.name}_splitw{j}",
                                engine=ins.engine,
                                ins=[],
                                outs=[],
                                sync_info=mybir.SyncInfo(on_wait=[w], on_update=[]),
                            )
                        )
                    ins.sync_info = mybir.SyncInfo(
                        on_wait=[waits[-1]], on_update=list(si.on_update)
                    )
                    changed = True
                newlist.append(ins)
            if changed:
                blk.instructions = newlist


def _host_constants(Wg1, bg1, wg2, bg2):
    """Build the folded block-diag layer-1 weights, relu biases and the
    signed reduce matrices."""
    aw = np.abs(wg2)  # [H, 3]
    sw = np.sign(wg2).astype(np.float32)
    kk = np.arange(32)

    bd = np.zeros((H, 96, 128), np.float32)
    rb = np.zeros((128, H), np.float32)
    lr = np.zeros((H, 4, 128, 128), np.float32)
    for c in range(3):
        for s in range(3):
            # bd[h, 3kk+c, 4kk+s] = |wg2[h,s]| * Wg1[h,c,s]
            bd[:, 3 * kk + c, 4 * kk + s] = aw[:, s : s + 1] * Wg1[:, c, s : s + 1]
    for s in range(3):
        rb[4 * kk + s, :] = (aw[:, s] * bg1[:, s])[np.newaxis, :]
        for c4 in range(4):
            lr[:, c4, 4 * kk + s, 32 * c4 + kk] = sw[:, s : s + 1]
    return bd, rb, lr


def kernel(**inputs):
    out, _ = _run(inputs, trace=False)
    return out


def kernel_traced(**inputs):
    return _run(inputs, trace=True)


def _run(inputs, trace=False):
    from concourse.bass_utils import run_bass_kernel_spmd

    X = {k: np.asarray(v, dtype=np.float32) for k, v in inputs.items()}
    Y = X["Y_lift"]          # [B, N, D]
    XP = X["X_pairs"]        # [B, N, N, 3]
    PQ = X["presence_q"]     # [B, N]
    PK = X["presence_k"]     # [B, N]

    bd, rb, lr = _host_constants(X["Wg1"], X["bg1"], X["wg2"], X["bg2"])

    wq = np.ascontiguousarray(X["Wq"] / 16.0)
    bq = np.ascontiguousarray((X["bq"] / 16.0).reshape(1, D))
    wk, bk = X["Wk"], X["bk"].reshape(1, D)
    wv, bv = X["Wv"], X["bv"].reshape(1, D)
    wo, bo = X["Wo"], X["bo"].reshape(1, D)

    if "nc" not in _CACHE:
        _CACHE["nc"] = _build_program()
    nc = _CACHE["nc"]

    in_maps = []
    for core in range(NCORES):
        b, half = core // 2, core % 2
        rows = slice(half * R, half * R + R)
        in_maps.append(
            {
                "y": np.ascontiguousarray(Y[b]),
                "yq": np.ascontiguousarray(Y[b, rows]),
                "xp": np.ascontiguousarray(XP[b, rows].reshape(R, 3 * N)),
                "pkc": np.ascontiguousarray(PK[b].reshape(8, 128).T),
                "pqr": np.ascontiguousarray(PQ[b, rows].reshape(1, R)),
                "pqcr": np.ascontiguousarray(1.0 - PQ[b, rows].reshape(1, R)),
                "wq": wq,
                "wk": np.ascontiguousarray(wk),
                "wv": np.ascontiguousarray(wv),
                "wo": np.ascontiguousarray(wo),
                "bq": bq,
                "bk": np.ascontiguousarray(bk),
                "bv": np.ascontiguousarray(bv),
                "bo": np.ascontiguousarray(bo),
                "bd": bd,
                "rb": rb,
                "lr": lr,
            }
        )

    res = run_bass_kernel_spmd(
        nc, in_maps, core_ids=list(range(NCORES)), trace=trace
    )
    out = np.empty((B, N, D), np.float32)
    for core in range(NCORES):
        b, half = core // 2, core % 2
        out[b, half * R : half * R + R] = res.results[core]["o"]
    return out, res



# revision 4
# speedup vs baseline: 2.2424x; 2.2424x over previous
"""Trainium2 Bass kernel for nn_EqvSelfAttention (B=4, N=1024, D=256, H=8).

Sharding: data-parallel over (batch b, query-half) -> 8 cores.
Each core computes all 8 heads for its 512 query rows against all 1024 keys.

Math notes (vs reference):
  * 1/sqrt(D)=1/16 folded into Wq (exact power of two).
  * Per-head location-bias MLP: loc_h = sum_d wg2[h,d]*relu(hid_hd) + bg2[h].
    - |wg2| folded into layer-1 weights/bias => z'_hd; sign applied in the
      PE "reduce" matmul that accumulates loc directly onto the content
      logits in PSUM (transposed layout [key, query]).
    - bg2 dropped: constant across keys => softmax-invariant.
  * Softmax computed without max subtraction (logits are O(+-6), exp is safe
    in fp32). Key presence mask folded into V'' = [pk*V | pk]; the 33rd
    column of the A@V'' matmul yields the softmax denominator Z.
  * Absent queries (pq=0) produce uniform attention over ALL keys in the
    reference => Oh = mean(V). Handled by blending with mean(V) after.
  * All heavy matmuls run in bf16 (operands); accumulation stays fp32 in
    PSUM. fp32 matmuls cost 2 half-rate PE passes (4x bf16 cost), so bf16
    cuts PE time ~4x; abs error lands ~1e-3 vs the 2e-2 gate.
"""

import sys
import numpy as np

sys.path.insert(0, "/opt/trn_rl_repo")

B, N, D, H, DH = 4, 1024, 256, 8, 32
R = 512  # query rows per core
NCORES = 8

_CACHE = {}


def _build_program():
    from contextlib import ExitStack

    from concourse import bass, mybir
    import concourse.tile as tile
    from concourse.masks import make_identity

    f32 = mybir.dt.float32
    bf16 = mybir.dt.bfloat16
    AF = mybir.ActivationFunctionType
    OP = mybir.AluOpType
    ds = bass.ds

    nc = bass.Bass("TRN2", target_bir_lowering=False, debug=False)

    # ---- I/O declarations (order matters for the PJRT call) ----
    d_y = nc.declare_dram_parameter("y", [N, D], f32, isOutput=False)
    d_yq = nc.declare_dram_parameter("yq", [R, D], f32, isOutput=False)
    d_xp = nc.declare_dram_parameter("xp", [R, 3 * N], f32, isOutput=False)
    d_pkc = nc.declare_dram_parameter("pkc", [128, 8], f32, isOutput=False)
    d_pqr = nc.declare_dram_parameter("pqr", [1, R], f32, isOutput=False)
    d_pqcr = nc.declare_dram_parameter("pqcr", [1, R], f32, isOutput=False)
    d_wq = nc.declare_dram_parameter("wq", [D, D], bf16, isOutput=False)
    d_wk = nc.declare_dram_parameter("wk", [D, D], bf16, isOutput=False)
    d_wv = nc.declare_dram_parameter("wv", [D, D], bf16, isOutput=False)
    d_wo = nc.declare_dram_parameter("wo", [D, D], f32, isOutput=False)
    d_bq = nc.declare_dram_parameter("bq", [1, D], bf16, isOutput=False)
    d_bk = nc.declare_dram_parameter("bk", [1, D], bf16, isOutput=False)
    d_bv = nc.declare_dram_parameter("bv", [1, D], bf16, isOutput=False)
    d_bo = nc.declare_dram_parameter("bo", [1, D], f32, isOutput=False)
    d_bd = nc.declare_dram_parameter("bd", [H, 96, 128], bf16, isOutput=False)
    d_rb = nc.declare_dram_parameter("rb", [128, H], f32, isOutput=False)
    d_lr = nc.declare_dram_parameter("lr", [H, 4, 128, 128], bf16, isOutput=False)
    d_o = nc.declare_dram_parameter("o", [R, D], f32, isOutput=True)

    with tile.TileContext(nc) as tc:
        with ExitStack() as ctx:
            consts = ctx.enter_context(tc.tile_pool(name="consts", bufs=1))
            persist = ctx.enter_context(tc.tile_pool(name="persist", bufs=1))

            # ---------- constants ----------
            ident = consts.tile([128, 128], f32)
            make_identity(nc, ident)
            ones512b = consts.tile([1, 512], bf16)
            nc.vector.memset(ones512b, 1.0)
            ones128b = consts.tile([1, 128], bf16)
            nc.vector.memset(ones128b, 1.0)
            ones128f = consts.tile([1, 128], f32)
            nc.vector.memset(ones128f, 1.0)
            inv1024c = consts.tile([128, 1], bf16)
            nc.vector.memset(inv1024c, 1.0 / 1024.0)

            wqs = consts.tile([128, 2, D], bf16)
            nc.sync.dma_start(wqs, d_wq[:, :].rearrange("(t p) d -> p t d", p=128))
            wks = consts.tile([128, 2, D], bf16)
            nc.sync.dma_start(wks, d_wk[:, :].rearrange("(t p) d -> p t d", p=128))
            wvs = consts.tile([128, 2, D], bf16)
            nc.sync.dma_start(wvs, d_wv[:, :].rearrange("(t p) d -> p t d", p=128))
            wos = consts.tile([128, 2, D], f32)
            nc.sync.dma_start(wos, d_wo[:, :].rearrange("(t p) d -> p t d", p=128))
            bqs = consts.tile([1, D], bf16)
            nc.sync.dma_start(bqs, d_bq[:, :])
            bks = consts.tile([1, D], bf16)
            nc.sync.dma_start(bks, d_bk[:, :])
            bvs = consts.tile([1, D], bf16)
            nc.sync.dma_start(bvs, d_bv[:, :])
            bos = consts.tile([1, D], f32)
            nc.sync.dma_start(bos, d_bo[:, :])
            bdsb = consts.tile([96, H, 128], bf16)
            nc.sync.dma_start(bdsb, d_bd[:, :, :].rearrange("h p m -> p h m"))
            rbsb = consts.tile([128, H], f32)
            nc.sync.dma_start(rbsb, d_rb[:, :])
            lrsb = consts.tile([128, H, 4, 128], bf16)
            nc.sync.dma_start(lrsb, d_lr[:, :, :, :].rearrange("h c p m -> p h c m"))
            pkcs = consts.tile([128, 8], f32)
            nc.sync.dma_start(pkcs, d_pkc[:, :])
            pqs = consts.tile([1, R], f32)
            nc.sync.dma_start(pqs, d_pqr[:, :])
            pqcs = consts.tile([1, R], f32)
            nc.sync.dma_start(pqcs, d_pqcr[:, :])

            # ---------- persistent activations ----------
            ktsb = persist.tile([128, 2, N], bf16)     # K^T [dout, key]
            qtz = persist.tile([128, H, 512], bf16)    # per-head zero-padded Q^T
            vsb = persist.tile([128, 8, D], bf16)      # V [key, dout]
            v2sb = persist.tile([128, 8, H, 33], bf16)  # [pk*V_h | pk]
            vtsb = persist.tile([128, 2, R], f32)      # V^T of my rows
            mvt = persist.tile([128, 2], f32)          # mean_k V  (transposed col)
            xtall = persist.tile([128, 8, 4, 512], bf16)  # Xp^T (96 rows used)
            otsb = persist.tile([128, 2, R], f32)      # O^T accumulator
            pqcb = persist.tile([128, R], f32)         # (1-pq) replicated rows

            nc.gpsimd.memset(qtz, 0.0)

            # ---------- phase A: Y^T and projections ----------
            with tc.tile_pool(name="ph_a", bufs=1) as pha, \
                 tc.tile_pool(name="ps_a", bufs=2, space="PSUM") as psa:
                ysb = pha.tile([128, 8, D], f32)
                nc.sync.dma_start(ysb, d_y[:, :].rearrange("(t p) d -> p t d", p=128))
                ysq = pha.tile([128, 4, D], f32)
                nc.sync.dma_start(ysq, d_yq[:, :].rearrange("(t p) d -> p t d", p=128))

                yt = pha.tile([128, 2, N], bf16)   # Y^T full batch
                ytq = pha.tile([128, 2, R], bf16)  # Y^T my rows
                for dt_ in range(2):
                    for g in range(2):  # groups of 4 n-tiles
                        ps = psa.tile([128, 512], f32)
                        for j in range(4):
                            nt = g * 4 + j
                            nc.tensor.transpose(
                                ps[:, ds(128 * j, 128)],
                                ysb[:, nt, ds(128 * dt_, 128)],
                                ident,
                            )
                        nc.vector.tensor_copy(yt[:, dt_, ds(512 * g, 512)], ps)
                    ps = psa.tile([128, 512], f32)
                    for j in range(4):
                        nc.tensor.transpose(
                            ps[:, ds(128 * j, 128)],
                            ysq[:, j, ds(128 * dt_, 128)],
                            ident,
                        )
                    nc.vector.tensor_copy(ytq[:, dt_], ps)

                qtsb = pha.tile([128, 2, R], bf16)
                # Q^T (scaled Wq), K^T, V, V^T projections
                for dt_ in range(2):
                    ps = psa.tile([128, 512], f32)
                    for k_ in range(2):
                        nc.tensor.matmul(
                            ps, wqs[:, k_, ds(128 * dt_, 128)], ytq[:, k_],
                            start=(k_ == 0), stop=False,
                        )
                    nc.tensor.matmul(
                        ps, bqs[0:1, ds(128 * dt_, 128)], ones512b,
                        start=False, stop=True,
                    )
                    nc.vector.tensor_copy(qtsb[:, dt_], ps)

                    for half in range(2):
                        ps = psa.tile([128, 512], f32)
                        for k_ in range(2):
                            nc.tensor.matmul(
                                ps, wks[:, k_, ds(128 * dt_, 128)],
                                yt[:, k_, ds(512 * half, 512)],
                                start=(k_ == 0), stop=False,
                            )
                        nc.tensor.matmul(
                            ps, bks[0:1, ds(128 * dt_, 128)], ones512b,
                            start=False, stop=True,
                        )
                        nc.vector.tensor_copy(ktsb[:, dt_, ds(512 * half, 512)], ps)

                    ps = psa.tile([128, 512], f32)
                    for k_ in range(2):
                        nc.tensor.matmul(
                            ps, wvs[:, k_, ds(128 * dt_, 128)], ytq[:, k_],
                            start=(k_ == 0), stop=False,
                        )
                    nc.tensor.matmul(
                        ps, bvs[0:1, ds(128 * dt_, 128)], ones512b,
                        start=False, stop=True,
                    )
                    nc.vector.tensor_copy(vtsb[:, dt_], ps)

                for nt in range(8):
                    ps = psa.tile([128, 256], f32)
                    for k_ in range(2):
                        nc.tensor.matmul(
                            ps, yt[:, k_, ds(128 * nt, 128)], wvs[:, k_],
                            start=(k_ == 0), stop=False,
                        )
                    nc.tensor.matmul(ps, ones128b, bvs, start=False, stop=True)
                    nc.vector.tensor_copy(vsb[:, nt], ps)

                # per-head zero-padded Q^T slices (keeps content matmuls K=128)
                for h in range(H):
                    base = 32 * (h % 4)
                    nc.vector.tensor_copy(
                        qtz[ds(base, 32), h], qtsb[ds(base, 32), h // 4]
                    )

                # V'' = [pk * V_h | pk]
                for nt in range(8):
                    nc.vector.tensor_scalar(
                        v2sb[:, nt, :, 0:32],
                        vsb[:, nt].rearrange("p (h d) -> p h d", h=H),
                        pkcs[:, nt : nt + 1],
                        None,
                        op0=OP.mult,
                    )
                    nc.vector.tensor_copy(
                        v2sb[:, nt, :, 32:33],
                        pkcs[:, nt : nt + 1].to_broadcast((128, H, 1)),
                    )

                # mean_k V (transposed): mvt[d] = sum_n V[n, d] / 1024
                psmv = psa.tile([128, 2], f32)
                for dt_ in range(2):
                    for nt in range(8):
                        nc.tensor.matmul(
                            psmv[:, dt_ : dt_ + 1],
                            vsb[:, nt, ds(128 * dt_, 128)],
                            inv1024c,
                            start=(nt == 0), stop=(nt == 7),
                        )
                nc.vector.tensor_copy(mvt, psmv)

            # ---------- phase B0: transpose X_pairs ----------
            with tc.tile_pool(name="xp_in", bufs=2) as xpin, \
                 tc.tile_pool(name="ps_t", bufs=2, space="PSUM") as pst:
                for kt in range(8):
                    xt_in = xpin.tile([128, 4, 384], f32)
                    for qt in range(4):
                        nc.sync.dma_start(
                            xt_in[:, qt],
                            d_xp[ds(128 * qt, 128), ds(384 * kt, 384)],
                        )
                    for cp in range(2):  # chunk pairs
                        ps = pst.tile([128, 2, 512], f32)
                        for ci in range(2):
                            for qt in range(4):
                                nc.tensor.transpose(
                                    ps[0:96, ci, ds(128 * qt, 128)],
                                    xt_in[:, qt, ds(96 * (2 * cp + ci), 96)],
                                    ident,
                                )
                        if cp % 2 == 0:
                            nc.scalar.copy(
                                xtall[0:96, kt, ds(2 * cp, 2)], ps[0:96]
                            )
                        else:
                            nc.vector.tensor_copy(
                                xtall[0:96, kt, ds(2 * cp, 2)], ps[0:96]
                            )

            # ---------- phase B1: attention main loop ----------
            with tc.tile_pool(name="ps_ct", bufs=2, space="PSUM") as psct, \
                 tc.tile_pool(name="ps_z", bufs=2, space="PSUM") as psz, \
                 tc.tile_pool(name="ps_av", bufs=2, space="PSUM") as psav, \
                 tc.tile_pool(name="rz_p", bufs=2) as rzp, \
                 tc.tile_pool(name="et_p", bufs=2) as etp, \
                 tc.tile_pool(name="fin_p", bufs=2) as finp:
                # replicate (1-pq) across partitions via a K=1 outer product
                psb = psct.tile([128, 512], f32, name="psbc", tag="ct")
                nc.tensor.matmul(psb, ones128f, pqcs, start=True, stop=True)
                nc.vector.tensor_copy(pqcb, psb)
                for h in range(H):
                    av = psav.tile([128, 512], f32)
                    for kt in range(8):
                        ct = psct.tile([128, 512], f32, name="ct", tag="ct")
                        nc.tensor.matmul(
                            ct,
                            ktsb[:, h // 4, ds(128 * kt, 128)],
                            qtz[:, h],
                            start=True, stop=False,
                        )
                        rzs = []
                        for cp in range(2):
                            zps = psz.tile([128, 2, 512], f32)
                            for ci in range(2):
                                nc.tensor.matmul(
                                    zps[:, ci], bdsb[:, h],
                                    xtall[0:96, kt, 2 * cp + ci],
                                    start=True, stop=True,
                                )
                            rz = rzp.tile([128, 2, 512], bf16)
                            if cp % 2 == 0:
                                nc.scalar.activation(
                                    rz, zps, AF.Relu, bias=rbsb[:, h : h + 1]
                                )
                            else:
                                nc.vector.tensor_scalar(
                                    rz, zps, rbsb[:, h : h + 1], 0.0,
                                    op0=OP.add, op1=OP.max,
                                )
                            rzs.append(rz)
                        for c4 in range(4):
                            nc.tensor.matmul(
                                ct, lrsb[:, h, c4], rzs[c4 // 2][:, c4 % 2],
                                start=False, stop=(c4 == 3),
                            )
                        et = etp.tile([128, 512], bf16)
                        nc.scalar.activation(et, ct, AF.Exp)
                        nc.tensor.matmul(
                            av[0:33], v2sb[:, kt, h], et,
                            start=(kt == 0), stop=(kt == 7),
                        )
                    # finalize head h
                    rec = finp.tile([1, 512], f32)
                    nc.vector.reciprocal(rec, av[32:33])
                    rpq = finp.tile([1, 512], f32)
                    nc.vector.tensor_mul(rpq, rec, pqs)
                    nc.tensor.matmul(
                        av[64:96], ones128f[0:1, 0:32], rpq, start=True, stop=True
                    )
                    rpqs = finp.tile([32, 512], f32)
                    nc.vector.tensor_copy(rpqs, av[64:96])
                    t2 = finp.tile([32, 512], f32)
                    nc.vector.tensor_mul(t2, av[0:32], rpqs)
                    mv0 = finp.tile([32, 1], f32)
                    nc.vector.tensor_copy(
                        mv0, mvt[ds(32 * (h % 4), 32), h // 4 : h // 4 + 1]
                    )
                    t3 = finp.tile([32, 512], f32)
                    nc.vector.tensor_scalar(
                        t3, pqcb[0:32], mv0, None, op0=OP.mult
                    )
                    t4 = finp.tile([32, 512], f32)
                    nc.vector.tensor_add(t4, t2, t3)
                    vt0 = finp.tile([32, 512], f32)
                    nc.vector.tensor_copy(vt0, vtsb[ds(32 * (h % 4), 32), h // 4])
                    nc.vector.tensor_add(
                        otsb[ds(32 * (h % 4), 32), h // 4], t4, vt0
                    )

            # ---------- phase C: O = O + relu(O @ Wo + bo) ----------
            with tc.tile_pool(name="ps_o", bufs=2, space="PSUM") as pso, \
                 tc.tile_pool(name="o_p", bufs=2) as op_:
                for j in range(4):
                    pso1 = pso.tile([128, 256], f32)
                    for dt_ in range(2):
                        nc.tensor.transpose(
                            pso1[:, ds(128 * dt_, 128)],
                            otsb[:, dt_, ds(128 * j, 128)],
                            ident,
                        )
                    oj = op_.tile([128, 256], f32)
                    nc.vector.tensor_copy(oj, pso1)

                    pso2 = pso.tile([128, 256], f32)
                    for dt_ in range(2):
                        nc.tensor.matmul(
                            pso2, otsb[:, dt_, ds(128 * j, 128)], wos[:, dt_],
                            start=(dt_ == 0), stop=False,
                        )
                    nc.tensor.matmul(pso2, ones128f, bos, start=False, stop=True)
                    r2 = op_.tile([128, 256], f32)
                    nc.scalar.activation(r2, pso2, AF.Relu)
                    ofin = op_.tile([128, 256], f32)
                    nc.vector.tensor_add(ofin, oj, r2)
                    nc.sync.dma_start(d_o[ds(128 * j, 128), :], ofin)

    _split_multiwait(nc, mybir)
    return nc


def _split_multiwait(nc, mybir):
    """This walrus build only encodes ONE sem-wait per instruction; Tile's
    tail drain carries several. Split extras onto preceding NoOps."""
    for f in nc.m.functions:
        for blk in f.blocks:
            insts = list(blk.instructions)
            changed = False
            newlist = []
            for ins in insts:
                si = ins.sync_info
                if si is not None and len(si.on_wait) > 1:
                    waits = list(si.on_wait)
                    for j, w in enumerate(waits[:-1]):
                        newlist.append(
                            mybir.InstNoOp(
                                name=f"{ins.name}_splitw{j}",
                                engine=ins.engine,
                                ins=[],
                                outs=[],
                                sync_info=mybir.SyncInfo(on_wait=[w], on_update=[]),
                            )
                        )
                    ins.sync_info = mybir.SyncInfo(
                        on_wait=[waits[-1]], on_update=list(si.on_update)
                    )
                    changed = True
                newlist.append(ins)
            if changed:
                blk.instructions = newlist


def _host_constants(Wg1, bg1, wg2, bg2):
    """Build the folded block-diag layer-1 weights, relu biases and the
    signed reduce matrices."""
    aw = np.abs(wg2)  # [H, 3]
    sw = np.sign(wg2).astype(np.float32)
    kk = np.arange(32)

    bd = np.zeros((H, 96, 128), np.float32)
    rb = np.zeros((128, H), np.float32)
    lr = np.zeros((H, 4, 128, 128), np.float32)
    for c in range(3):
        for s in range(3):
            # bd[h, 3kk+c, 4kk+s] = |wg2[h,s]| * Wg1[h,c,s]
            bd[:, 3 * kk + c, 4 * kk + s] = aw[:, s : s + 1] * Wg1[:, c, s : s + 1]
    for s in range(3):
        rb[4 * kk + s, :] = (aw[:, s] * bg1[:, s])[np.newaxis, :]
        for c4 in range(4):
            lr[:, c4, 4 * kk + s, 32 * c4 + kk] = sw[:, s : s + 1]
    return bd, rb, lr


def kernel(**inputs):
    out, _ = _run(inputs, trace=False)
    return out


def kernel_traced(**inputs):
    return _run(inputs, trace=True)


def _run(inputs, trace=False):
    import ml_dtypes
    from concourse.bass_utils import run_bass_kernel_spmd

    bf = ml_dtypes.bfloat16

    X = {k: np.asarray(v, dtype=np.float32) for k, v in inputs.items()}
    Y = X["Y_lift"]          # [B, N, D]
    XP = X["X_pairs"]        # [B, N, N, 3]
    PQ = X["presence_q"]     # [B, N]
    PK = X["presence_k"]     # [B, N]

    bd, rb, lr = _host_constants(X["Wg1"], X["bg1"], X["wg2"], X["bg2"])
    bd16 = np.ascontiguousarray(bd.astype(bf))
    lr16 = np.ascontiguousarray(lr.astype(bf))

    wq = np.ascontiguousarray((X["Wq"] / 16.0).astype(bf))
    bq = np.ascontiguousarray((X["bq"] / 16.0).reshape(1, D).astype(bf))
    wk = np.ascontiguousarray(X["Wk"].astype(bf))
    bk = np.ascontiguousarray(X["bk"].reshape(1, D).astype(bf))
    wv = np.ascontiguousarray(X["Wv"].astype(bf))
    bv = np.ascontiguousarray(X["bv"].reshape(1, D).astype(bf))
    wo = np.ascontiguousarray(X["Wo"])
    bo = np.ascontiguousarray(X["bo"].reshape(1, D))

    if "nc" not in _CACHE:
        _CACHE["nc"] = _build_program()
    nc = _CACHE["nc"]

    in_maps = []
    for core in range(NCORES):
        b, half = core // 2, core % 2
        rows = slice(half * R, half * R + R)
        in_maps.append(
            {
                "y": np.ascontiguousarray(Y[b]),
                "yq": np.ascontiguousarray(Y[b, rows]),
                "xp": np.ascontiguousarray(XP[b, rows].reshape(R, 3 * N)),
                "pkc": np.ascontiguousarray(PK[b].reshape(8, 128).T),
                "pqr": np.ascontiguousarray(PQ[b, rows].reshape(1, R)),
                "pqcr": np.ascontiguousarray(1.0 - PQ[b, rows].reshape(1, R)),
                "wq": wq,
                "wk": wk,
                "wv": wv,
                "wo": wo,
                "bq": bq,
                "bk": bk,
                "bv": bv,
                "bo": bo,
                "bd": bd16,
                "rb": rb,
                "lr": lr16,
            }
        )

    res = run_bass_kernel_spmd(
        nc, in_maps, core_ids=list(range(NCORES)), trace=trace
    )
    out = np.empty((B, N, D), np.float32)
    for core in range(NCORES):
        b, half = core // 2, core % 2
        out[b, half * R : half * R + R] = res.results[core]["o"]
    return out, res


# revision 16
# speedup vs baseline: 2.6331x; 1.1742x over previous
"""Trainium2 Bass kernel for nn_EqvSelfAttention (B=4, N=1024, D=256, H=8).

Sharding: data-parallel over (batch b, query-half) -> 8 cores.
Each core computes all 8 heads for its 512 query rows against all 1024 keys.

Math notes (vs reference):
  * 1/sqrt(D)=1/16 folded into Wq (exact power of two).
  * Per-head location-bias MLP: loc_h = sum_d wg2[h,d]*relu(hid_hd) + bg2[h].
    - |wg2| folded into layer-1 weights/bias => z'_hd; sign applied in the
      PE "reduce" matmul that accumulates loc directly onto the content
      logits in PSUM (transposed layout [key, query]).
    - bg2 dropped: constant across keys => softmax-invariant.
  * Softmax computed without max subtraction (logits are O(+-6), exp is safe
    in fp32). Key presence mask folded into V'' = [pk*V | pk]; the 33rd
    column of the A@V'' matmul yields the softmax denominator Z.
  * Absent queries (pq=0) produce uniform attention over ALL keys in the
    reference => Oh = mean(V). Handled by blending with mean(V) after.
  * All heavy matmuls run in bf16 (operands); accumulation stays fp32 in
    PSUM.
  * The A@V matmul for key-tile kt is emitted one iteration late so the PE
    never waits on the exp() of the current tile.
  * Per-head softmax normalization is deferred: unnormalized head outputs
    and denominators are collected, then normalized for all heads at once
    (one ACT reciprocal on [8,512], one broadcast matmul per query-half).
"""

import sys
import numpy as np

sys.path.insert(0, "/opt/trn_rl_repo")

B, N, D, H, DH = 4, 1024, 256, 8, 32
R = 512  # query rows per core
NCORES = 8

_CACHE = {}


def _build_program():
    from contextlib import ExitStack

    from concourse import bass, mybir
    import concourse.tile as tile
    from concourse.masks import make_identity

    f32 = mybir.dt.float32
    bf16 = mybir.dt.bfloat16
    AF = mybir.ActivationFunctionType
    OP = mybir.AluOpType
    ds = bass.ds

    nc = bass.Bass("TRN2", target_bir_lowering=False, debug=False)

    # ---- I/O declarations (order matters for the PJRT call) ----
    d_y = nc.declare_dram_parameter("y", [N, D], f32, isOutput=False)
    d_yq = nc.declare_dram_parameter("yq", [R, D], f32, isOutput=False)
    d_xp = nc.declare_dram_parameter("xp", [R, 3 * N], f32, isOutput=False)
    d_pkc = nc.declare_dram_parameter("pkc", [128, 8], f32, isOutput=False)
    d_pqr = nc.declare_dram_parameter("pqr", [1, R], f32, isOutput=False)
    d_pqcr = nc.declare_dram_parameter("pqcr", [1, R], f32, isOutput=False)
    d_wq = nc.declare_dram_parameter("wq", [D, D], bf16, isOutput=False)
    d_wk = nc.declare_dram_parameter("wk", [D, D], bf16, isOutput=False)
    d_wv = nc.declare_dram_parameter("wv", [D, D], bf16, isOutput=False)
    d_wo = nc.declare_dram_parameter("wo", [D, D], f32, isOutput=False)
    d_bq = nc.declare_dram_parameter("bq", [1, D], bf16, isOutput=False)
    d_bk = nc.declare_dram_parameter("bk", [1, D], bf16, isOutput=False)
    d_bv = nc.declare_dram_parameter("bv", [1, D], bf16, isOutput=False)
    d_bo = nc.declare_dram_parameter("bo", [1, D], f32, isOutput=False)
    d_bd = nc.declare_dram_parameter("bd", [H, 96, 128], bf16, isOutput=False)
    d_rb = nc.declare_dram_parameter("rb", [128, H], f32, isOutput=False)
    d_lr = nc.declare_dram_parameter("lr", [H, 4, 128, 128], bf16, isOutput=False)
    d_e8 = nc.declare_dram_parameter("e8", [8, 2, 128], bf16, isOutput=False)
    d_eg = nc.declare_dram_parameter("eg", [1, 64], f32, isOutput=False)
    d_o = nc.declare_dram_parameter("o", [R, D], f32, isOutput=True)

    with tile.TileContext(nc) as tc:
        with ExitStack() as ctx:
            consts = ctx.enter_context(tc.tile_pool(name="consts", bufs=1))
            persist = ctx.enter_context(tc.tile_pool(name="persist", bufs=1))

            # ---------- DMAs first, in consumption order ----------
            # (sync HWDGE drains FIFO: phase-A inputs, then X_pairs prefetch,
            # then the weights needed later)
            ysb = persist.tile([128, 8, D], f32)
            nc.sync.dma_start(ysb, d_y[:, :].rearrange("(t p) d -> p t d", p=128))
            ysq = persist.tile([128, 4, D], f32)
            nc.sync.dma_start(ysq, d_yq[:, :].rearrange("(t p) d -> p t d", p=128))

            wqs = consts.tile([128, 2, D], bf16)
            nc.sync.dma_start(wqs, d_wq[:, :].rearrange("(t p) d -> p t d", p=128))
            wks = consts.tile([128, 2, D], bf16)
            nc.sync.dma_start(wks, d_wk[:, :].rearrange("(t p) d -> p t d", p=128))
            wvs = consts.tile([128, 2, D], bf16)
            nc.sync.dma_start(wvs, d_wv[:, :].rearrange("(t p) d -> p t d", p=128))
            bqs = consts.tile([1, D], bf16)
            nc.sync.dma_start(bqs, d_bq[:, :])
            bks = consts.tile([1, D], bf16)
            nc.sync.dma_start(bks, d_bk[:, :])
            bvs = consts.tile([1, D], bf16)
            nc.sync.dma_start(bvs, d_bv[:, :])
            pkcs = consts.tile([128, 8], f32)
            nc.sync.dma_start(pkcs, d_pkc[:, :])
            pqs = consts.tile([1, R], f32)
            nc.sync.dma_start(pqs, d_pqr[:, :])
            pqcs = consts.tile([1, R], f32)
            nc.sync.dma_start(pqcs, d_pqcr[:, :])

            # full X_pairs prefetch (6.3 MB) — overlaps phase A compute
            xpall = persist.tile([128, 8, 4, 384], f32)
            for kt in range(8):
                nc.sync.dma_start(
                    xpall[:, kt],
                    d_xp[:, ds(384 * kt, 384)].rearrange(
                        "(t p) c -> p t c", p=128
                    ),
                )

            bdsb = consts.tile([96, H, 128], bf16)
            nc.sync.dma_start(bdsb, d_bd[:, :, :].rearrange("h p m -> p h m"))
            rbsb = consts.tile([128, H], f32)
            nc.sync.dma_start(rbsb, d_rb[:, :])
            lrsb = consts.tile([128, H, 4, 128], bf16)
            nc.sync.dma_start(lrsb, d_lr[:, :, :, :].rearrange("h c p m -> p h c m"))
            e8sb = consts.tile([8, 2, 128], bf16)
            nc.sync.dma_start(e8sb, d_e8[:, :, :])
            e8g = consts.tile([1, 64], f32)
            nc.sync.dma_start(e8g, d_eg[:, :])
            wos = consts.tile([128, 2, D], f32)
            nc.sync.dma_start(wos, d_wo[:, :].rearrange("(t p) d -> p t d", p=128))
            bos = consts.tile([1, D], f32)
            nc.sync.dma_start(bos, d_bo[:, :])

            # ---------- constants ----------
            ident = consts.tile([128, 128], f32)
            make_identity(nc, ident)
            ones512b = consts.tile([1, 512], bf16)
            nc.vector.memset(ones512b, 1.0)
            ones128b = consts.tile([1, 128], bf16)
            nc.vector.memset(ones128b, 1.0)
            ones128f = consts.tile([1, 128], f32)
            nc.vector.memset(ones128f, 1.0)
            inv1024c = consts.tile([128, 1], bf16)
            nc.vector.memset(inv1024c, 1.0 / 1024.0)

            # ---------- persistent activations ----------
            ktsb = persist.tile([128, 2, N], bf16)     # K^T [dout, key]
            qtz = persist.tile([128, H, 512], bf16)    # per-head zero-padded Q^T
            vsb = persist.tile([128, 8, D], bf16)      # V [key, dout]
            v2sb = persist.tile([128, 8, H, 33], bf16)  # [pk*V_h | pk]
            vtsb = persist.tile([128, 2, R], f32)      # V^T of my rows
            mvt = persist.tile([128, 2], f32)          # mean_k V  (transposed col)
            xtall = persist.tile([128, 8, 4, 512], bf16)  # Xp^T (96 rows used)
            otsb = persist.tile([128, 2, R], f32)      # O^T accumulator
            pqcb = persist.tile([128, R], f32)         # (1-pq) replicated rows
            vmv = persist.tile([128, 2, R], f32)       # V^T + (1-pq)*meanV
            ohu = persist.tile([128, 2, R], f32)       # unnormalized head outs
            pq8 = persist.tile([8, R], f32)            # pq replicated to 8 rows

            nc.gpsimd.memset(qtz, 0.0)

            # ---------- phase A: Y^T and projections ----------
            with tc.tile_pool(name="ph_a", bufs=1) as pha, \
                 tc.tile_pool(name="ps_a", bufs=2, space="PSUM") as psa:
                yt = pha.tile([128, 2, N], bf16)   # Y^T full batch
                ytq = pha.tile([128, 2, R], bf16)  # Y^T my rows
                for dt_ in range(2):
                    for g in range(2):  # groups of 4 n-tiles
                        ps = psa.tile([128, 512], f32)
                        for j in range(4):
                            nt = g * 4 + j
                            nc.tensor.transpose(
                                ps[:, ds(128 * j, 128)],
                                ysb[:, nt, ds(128 * dt_, 128)],
                                ident,
                            )
                        nc.vector.tensor_copy(yt[:, dt_, ds(512 * g, 512)], ps)
                    ps = psa.tile([128, 512], f32)
                    for j in range(4):
                        nc.tensor.transpose(
                            ps[:, ds(128 * j, 128)],
                            ysq[:, j, ds(128 * dt_, 128)],
                            ident,
                        )
                    nc.vector.tensor_copy(ytq[:, dt_], ps)

                qtsb = pha.tile([128, 2, R], bf16)
                # Q^T (scaled Wq), K^T, V, V^T projections
                for dt_ in range(2):
                    ps = psa.tile([128, 512], f32)
                    for k_ in range(2):
                        nc.tensor.matmul(
                            ps, wqs[:, k_, ds(128 * dt_, 128)], ytq[:, k_],
                            start=(k_ == 0), stop=False,
                        )
                    nc.tensor.matmul(
                        ps, bqs[0:1, ds(128 * dt_, 128)], ones512b,
                        start=False, stop=True,
                    )
                    nc.vector.tensor_copy(qtsb[:, dt_], ps)

                    for half in range(2):
                        ps = psa.tile([128, 512], f32)
                        for k_ in range(2):
                            nc.tensor.matmul(
                                ps, wks[:, k_, ds(128 * dt_, 128)],
                                yt[:, k_, ds(512 * half, 512)],
                                start=(k_ == 0), stop=False,
                            )
                        nc.tensor.matmul(
                            ps, bks[0:1, ds(128 * dt_, 128)], ones512b,
                            start=False, stop=True,
                        )
                        nc.vector.tensor_copy(ktsb[:, dt_, ds(512 * half, 512)], ps)

                    ps = psa.tile([128, 512], f32)
                    for k_ in range(2):
                        nc.tensor.matmul(
                            ps, wvs[:, k_, ds(128 * dt_, 128)], ytq[:, k_],
                            start=(k_ == 0), stop=False,
                        )
                    nc.tensor.matmul(
                        ps, bvs[0:1, ds(128 * dt_, 128)], ones512b,
                        start=False, stop=True,
                    )
                    nc.vector.tensor_copy(vtsb[:, dt_], ps)

                for nt in range(8):
                    ps = psa.tile([128, 256], f32)
                    for k_ in range(2):
                        nc.tensor.matmul(
                            ps, yt[:, k_, ds(128 * nt, 128)], wvs[:, k_],
                            start=(k_ == 0), stop=False,
                        )
                    nc.tensor.matmul(ps, ones128b, bvs, start=False, stop=True)
                    nc.vector.tensor_copy(vsb[:, nt], ps)

                # per-head zero-padded Q^T slices (keeps content matmuls K=128)
                for h in range(H):
                    base = 32 * (h % 4)
                    nc.vector.tensor_copy(
                        qtz[ds(base, 32), h], qtsb[ds(base, 32), h // 4]
                    )

                # V'' = [pk * V_h | pk]
                for nt in range(8):
                    nc.vector.tensor_scalar(
                        v2sb[:, nt, :, 0:32],
                        vsb[:, nt].rearrange("p (h d) -> p h d", h=H),
                        pkcs[:, nt : nt + 1],
                        None,
                        op0=OP.mult,
                    )
                    nc.vector.tensor_copy(
                        v2sb[:, nt, :, 32:33],
                        pkcs[:, nt : nt + 1].to_broadcast((128, H, 1)),
                    )

                # mean_k V (transposed): mvt[d] = sum_n V[n, d] / 1024
                psmv = psa.tile([128, 2], f32)
                for dt_ in range(2):
                    for nt in range(8):
                        nc.tensor.matmul(
                            psmv[:, dt_ : dt_ + 1],
                            vsb[:, nt, ds(128 * dt_, 128)],
                            inv1024c,
                            start=(nt == 0), stop=(nt == 7),
                        )
                nc.vector.tensor_copy(mvt, psmv)

            # ---------- phase B0: transpose X_pairs ----------
            with tc.tile_pool(name="ps_t", bufs=2, space="PSUM") as pst:
                for kt in range(8):
                    for cp in range(2):  # chunk pairs
                        ps = pst.tile([128, 2, 512], f32)
                        for ci in range(2):
                            for qt in range(4):
                                nc.tensor.transpose(
                                    ps[0:96, ci, ds(128 * qt, 128)],
                                    xpall[:, kt, qt, ds(96 * (2 * cp + ci), 96)],
                                    ident,
                                )
                        if cp % 2 == 0:
                            nc.scalar.copy(
                                xtall[0:96, kt, ds(2 * cp, 2)], ps[0:96]
                            )
                        else:
                            nc.vector.tensor_copy(
                                xtall[0:96, kt, ds(2 * cp, 2)], ps[0:96]
                            )

            # ---------- phase B1: attention main loop ----------
            with tc.tile_pool(name="ps_ct", bufs=2, space="PSUM") as psct, \
                 tc.tile_pool(name="ps_z", bufs=2, space="PSUM") as psz, \
                 tc.tile_pool(name="ps_av", bufs=1, space="PSUM") as psav, \
                 tc.tile_pool(name="rz_p", bufs=2) as rzp, \
                 tc.tile_pool(name="et_p", bufs=3) as etp, \
                 tc.tile_pool(name="fin_p", bufs=2) as finp:
                # replicate (1-pq) across partitions via a K=1 outer product
                psb = psct.tile([128, 512], f32, name="psbc", tag="ct")
                nc.tensor.matmul(psb, ones128f, pqcs, start=True, stop=True)
                nc.vector.tensor_copy(pqcb, psb)
                # replicate pq to 8 rows (for the deferred normalization)
                psq = psav.tile([128, 512], f32, name="psq", tag="zg")
                nc.tensor.matmul(psq[0:8], ones128f[0:1, 0:8], pqs,
                                 start=True, stop=True)
                nc.vector.tensor_copy(pq8, psq[0:8])
                # collector for the 8 per-head softmax denominators
                z8ps = psav.tile([128, 512], f32, name="z8ps", tag="zg")
                # vmv = V^T + (1-pq)*meanV, per dim-half
                for half in range(2):
                    tmp = finp.tile([128, 512], f32, tag="fin")
                    nc.vector.tensor_scalar(
                        tmp, pqcb, mvt[:, half : half + 1], None, op0=OP.mult
                    )
                    nc.vector.tensor_add(vmv[:, half], tmp, vtsb[:, half])

                for h in range(H):
                    av = psav.tile([128, 512], f32, tag="av")
                    pend = None  # noqa: semantic marker
                    for kt in range(8):
                        ct = psct.tile([128, 512], f32, name="ct", tag="ct")
                        nc.tensor.matmul(
                            ct,
                            ktsb[:, h // 4, ds(128 * kt, 128)],
                            qtz[:, h],
                            start=True, stop=False,
                        )
                        rzs = []
                        for cp in range(2):
                            zps = psz.tile([128, 2, 512], f32)
                            for ci in range(2):
                                nc.tensor.matmul(
                                    zps[:, ci], bdsb[:, h],
                                    xtall[0:96, kt, 2 * cp + ci],
                                    start=True, stop=True,
                                )
                            rz = rzp.tile([128, 2, 512], bf16)
                            if cp % 2 == 0:
                                nc.scalar.activation(
                                    rz, zps, AF.Relu, bias=rbsb[:, h : h + 1]
                                )
                            else:
                                nc.vector.tensor_scalar(
                                    rz, zps, rbsb[:, h : h + 1], 0.0,
                                    op0=OP.add, op1=OP.max,
                                )
                            rzs.append(rz)
                        for c4 in range(4):
                            nc.tensor.matmul(
                                ct, lrsb[:, h, c4], rzs[c4 // 2][:, c4 % 2],
                                start=False, stop=(c4 == 3),
                            )
                        # previous tile's A@V — PE already has et(kt-1) ready,
                        # so this never stalls on the current exp
                        if pend is not None:
                            nc.tensor.matmul(
                                av[0:33], v2sb[:, kt - 1, h], pend,
                                start=(kt == 1), stop=False,
                            )
                        et = etp.tile([128, 512], bf16)
                        nc.scalar.activation(et, ct, AF.Exp)
                        pend = et
                    nc.tensor.matmul(
                        av[0:33], v2sb[:, 7, h], pend, start=False, stop=True
                    )
                    # stash unnormalized result + denominator (32-aligned
                    # partition shifts only; arbitrary shifts are
                    # uncharacterized on HW)
                    nc.vector.tensor_copy(
                        ohu[ds(32 * (h % 4), 32), h // 4], av[0:32]
                    )
                    zrow = finp.tile([1, 512], f32, tag="zrow")
                    nc.vector.tensor_copy(zrow, av[32:33])
                    nc.tensor.matmul(
                        z8ps[0:8], e8g[0:1, ds(8 * h, 8)], zrow,
                        start=(h == 0), stop=(h == 7),
                    )

                # deferred normalization for all heads
                rec8 = finp.tile([8, 512], f32, tag="fin8")
                nc.vector.reciprocal(rec8, z8ps[0:8])
                rpq8 = finp.tile([8, 512], bf16, tag="fin8b")
                nc.vector.tensor_mul(rpq8, rec8, pq8)
                for half in range(2):
                    psb2 = psct.tile([128, 512], f32, name="psb2", tag="ct")
                    nc.tensor.matmul(
                        psb2, e8sb[:, half], rpq8, start=True, stop=True
                    )
                    rps = finp.tile([128, 512], f32, tag="fin")
                    nc.vector.tensor_copy(rps, psb2)
                    t_ = finp.tile([128, 512], f32, tag="fin")
                    nc.vector.tensor_mul(t_, ohu[:, half], rps)
                    nc.vector.tensor_add(otsb[:, half], t_, vmv[:, half])

            # ---------- phase C: O = O + relu(O @ Wo + bo) ----------
            with tc.tile_pool(name="ps_o", bufs=2, space="PSUM") as pso, \
                 tc.tile_pool(name="o_p", bufs=2) as op_:
                for j in range(4):
                    pso1 = pso.tile([128, 256], f32)
                    for dt_ in range(2):
                        nc.tensor.transpose(
                            pso1[:, ds(128 * dt_, 128)],
                            otsb[:, dt_, ds(128 * j, 128)],
                            ident,
                        )
                    oj = op_.tile([128, 256], f32)
                    nc.vector.tensor_copy(oj, pso1)

                    pso2 = pso.tile([128, 256], f32)
                    for dt_ in range(2):
                        nc.tensor.matmul(
                            pso2, otsb[:, dt_, ds(128 * j, 128)], wos[:, dt_],
                            start=(dt_ == 0), stop=False,
                        )
                    nc.tensor.matmul(pso2, ones128f, bos, start=False, stop=True)
                    r2 = op_.tile([128, 256], f32)
                    nc.scalar.activation(r2, pso2, AF.Relu)
                    ofin = op_.tile([128, 256], f32)
                    nc.vector.tensor_add(ofin, oj, r2)
                    nc.sync.dma_start(d_o[ds(128 * j, 128), :], ofin)

    _split_multiwait(nc, mybir)
    return nc


def _split_multiwait(nc, mybir):
    """This walrus build only encodes ONE sem-wait per instruction; Tile's
    tail drain carries several. Split extras onto preceding NoOps."""
    for f in nc.m.functions:
        for blk in f.blocks:
            insts = list(blk.instructions)
            changed = False
            newlist = []
            for ins in insts:
                si = ins.sync_info
                if si is not None and len(si.on_wait) > 1:
                    waits = list(si.on_wait)
                    for j, w in enumerate(waits[:-1]):
                        newlist.append(
                            mybir.InstNoOp(
                                name=f"{ins.name}_splitw{j}",
                                engine=ins.engine,
                                ins=[],
                                outs=[],
                                sync_info=mybir.SyncInfo(on_wait=[w], on_update=[]),
                            )
                        )
                    ins.sync_info = mybir.SyncInfo(
                        on_wait=[waits[-1]], on_update=list(si.on_update)
                    )
                    changed = True
                newlist.append(ins)
            if changed:
                blk.instructions = newlist


def _host_constants(Wg1, bg1, wg2, bg2):
    """Build the folded block-diag layer-1 weights, relu biases and the
    signed reduce matrices."""
    aw = np.abs(wg2)  # [H, 3]
    sw = np.sign(wg2).astype(np.float32)
    kk = np.arange(32)

    bd = np.zeros((H, 96, 128), np.float32)
    rb = np.zeros((128, H), np.float32)
    lr = np.zeros((H, 4, 128, 128), np.float32)
    for c in range(3):
        for s in range(3):
            # bd[h, 3kk+c, 4kk+s] = |wg2[h,s]| * Wg1[h,c,s]
            bd[:, 3 * kk + c, 4 * kk + s] = aw[:, s : s + 1] * Wg1[:, c, s : s + 1]
    for s in range(3):
        rb[4 * kk + s, :] = (aw[:, s] * bg1[:, s])[np.newaxis, :]
        for c4 in range(4):
            lr[:, c4, 4 * kk + s, 32 * c4 + kk] = sw[:, s : s + 1]
    # head -> output-row selector for the deferred normalization broadcast
    e8 = np.zeros((8, 2, 128), np.float32)
    for h in range(H):
        e8[h, h // 4, 32 * (h % 4) : 32 * (h % 4) + 32] = 1.0
    # per-head one-hot rows for the Z-gather matmul
    eg = np.eye(8, dtype=np.float32).reshape(1, 64)
    return bd, rb, lr, e8, eg


def kernel(**inputs):
    out, _ = _run(inputs, trace=False)
    return out


def kernel_traced(**inputs):
    return _run(inputs, trace=True)


def _run(inputs, trace=False):
    import ml_dtypes
    from concourse.bass_utils import run_bass_kernel_spmd

    bf = ml_dtypes.bfloat16

    X = {k: np.asarray(v, dtype=np.float32) for k, v in inputs.items()}
    Y = X["Y_lift"]          # [B, N, D]
    XP = X["X_pairs"]        # [B, N, N, 3]
    PQ = X["presence_q"]     # [B, N]
    PK = X["presence_k"]     # [B, N]

    bd, rb, lr, e8, eg = _host_constants(X["Wg1"], X["bg1"], X["wg2"], X["bg2"])
    bd16 = np.ascontiguousarray(bd.astype(bf))
    lr16 = np.ascontiguousarray(lr.astype(bf))
    e816 = np.ascontiguousarray(e8.astype(bf))

    wq = np.ascontiguousarray((X["Wq"] / 16.0).astype(bf))
    bq = np.ascontiguousarray((X["bq"] / 16.0).reshape(1, D).astype(bf))
    wk = np.ascontiguousarray(X["Wk"].astype(bf))
    bk = np.ascontiguousarray(X["bk"].reshape(1, D).astype(bf))
    wv = np.ascontiguousarray(X["Wv"].astype(bf))
    bv = np.ascontiguousarray(X["bv"].reshape(1, D).astype(bf))
    wo = np.ascontiguousarray(X["Wo"])
    bo = np.ascontiguousarray(X["bo"].reshape(1, D))

    if "nc" not in _CACHE:
        _CACHE["nc"] = _build_program()
    nc = _CACHE["nc"]

    in_maps = []
    for core in range(NCORES):
        b, half = core // 2, core % 2
        rows = slice(half * R, half * R + R)
        in_maps.append(
            {
                "y": np.ascontiguousarray(Y[b]),
                "yq": np.ascontiguousarray(Y[b, rows]),
                "xp": np.ascontiguousarray(XP[b, rows].reshape(R, 3 * N)),
                "pkc": np.ascontiguousarray(PK[b].reshape(8, 128).T),
                "pqr": np.ascontiguousarray(PQ[b, rows].reshape(1, R)),
                "pqcr": np.ascontiguousarray(1.0 - PQ[b, rows].reshape(1, R)),
                "wq": wq,
                "wk": wk,
                "wv": wv,
                "wo": wo,
                "bq": bq,
                "bk": bk,
                "bv": bv,
                "bo": bo,
                "bd": bd16,
                "rb": rb,
                "lr": lr16,
                "e8": e816,
                "eg": np.ascontiguousarray(eg),
            }
        )

    res = run_bass_kernel_spmd(
        nc, in_maps, core_ids=list(range(NCORES)), trace=trace
    )
    out = np.empty((B, N, D), np.float32)
    for core in range(NCORES):
        b, half = core // 2, core % 2
        out[b, half * R : half * R + R] = res.results[core]["o"]
    return out, res


# revision 23
# speedup vs baseline: 2.6861x; 1.0201x over previous
"""Trainium2 Bass kernel for nn_EqvSelfAttention (B=4, N=1024, D=256, H=8).

Sharding: data-parallel over (batch b, query-half) -> 8 cores.
Each core computes all 8 heads for its 512 query rows against all 1024 keys.

Math notes (vs reference):
  * 1/sqrt(D)=1/16 folded into Wq (exact power of two).
  * Per-head location-bias MLP: loc_h = sum_d wg2[h,d]*relu(hid_hd) + bg2[h].
    - |wg2| folded into layer-1 weights/bias => z'_hd; sign applied in the
      PE "reduce" matmul that accumulates loc directly onto the content
      logits in PSUM (transposed layout [key, query]).
    - bg2 dropped: constant across keys => softmax-invariant.
  * Softmax computed without max subtraction (logits are O(+-6), exp is safe
    in fp32). Key presence mask folded into V'' = [pk*V | pk]; the 33rd
    column of the A@V'' matmul yields the softmax denominator Z.
  * Absent queries (pq=0) produce uniform attention over ALL keys in the
    reference => Oh = mean(V). Handled by blending with mean(V) after.
  * All heavy matmuls run in bf16 (operands); accumulation stays fp32 in
    PSUM.
  * The A@V matmul for key-tile kt is emitted one iteration late so the PE
    never waits on the exp() of the current tile.
  * Per-head softmax normalization is deferred: unnormalized head outputs
    and denominators are collected, then normalized for all heads at once
    (one ACT reciprocal on [8,512], one broadcast matmul per query-half).
"""

import sys
import numpy as np

sys.path.insert(0, "/opt/trn_rl_repo")

B, N, D, H, DH = 4, 1024, 256, 8, 32
R = 512  # query rows per core
NCORES = 8

_CACHE = {}


def _build_program():
    from contextlib import ExitStack

    from concourse import bass, mybir
    import concourse.tile as tile
    from concourse.masks import make_identity

    f32 = mybir.dt.float32
    bf16 = mybir.dt.bfloat16
    AF = mybir.ActivationFunctionType
    OP = mybir.AluOpType
    ds = bass.ds

    nc = bass.Bass("TRN2", target_bir_lowering=False, debug=False)

    # ---- I/O declarations (order matters for the PJRT call) ----
    d_y = nc.declare_dram_parameter("y", [N, D], f32, isOutput=False)
    d_yq = nc.declare_dram_parameter("yq", [R, D], f32, isOutput=False)
    d_xp = nc.declare_dram_parameter("xp", [R, 3 * N], f32, isOutput=False)
    d_pkc = nc.declare_dram_parameter("pkc", [128, 8], f32, isOutput=False)
    d_pqr = nc.declare_dram_parameter("pqr", [1, R], f32, isOutput=False)
    d_pqcr = nc.declare_dram_parameter("pqcr", [1, R], f32, isOutput=False)
    d_wq = nc.declare_dram_parameter("wq", [D, D], bf16, isOutput=False)
    d_wk = nc.declare_dram_parameter("wk", [D, D], bf16, isOutput=False)
    d_wv = nc.declare_dram_parameter("wv", [D, D], bf16, isOutput=False)
    d_wo = nc.declare_dram_parameter("wo", [D, D], f32, isOutput=False)
    d_bq = nc.declare_dram_parameter("bq", [1, D], bf16, isOutput=False)
    d_bk = nc.declare_dram_parameter("bk", [1, D], bf16, isOutput=False)
    d_bv = nc.declare_dram_parameter("bv", [1, D], bf16, isOutput=False)
    d_bo = nc.declare_dram_parameter("bo", [1, D], f32, isOutput=False)
    d_bd = nc.declare_dram_parameter("bd", [H, 96, 128], bf16, isOutput=False)
    d_rb = nc.declare_dram_parameter("rb", [128, H], f32, isOutput=False)
    d_lr = nc.declare_dram_parameter("lr", [H, 4, 128, 128], bf16, isOutput=False)
    d_e8 = nc.declare_dram_parameter("e8", [8, 2, 128], bf16, isOutput=False)
    d_eg = nc.declare_dram_parameter("eg", [1, 64], f32, isOutput=False)
    d_o = nc.declare_dram_parameter("o", [R, D], f32, isOutput=True)

    with tile.TileContext(nc) as tc:
        with ExitStack() as ctx:
            consts = ctx.enter_context(tc.tile_pool(name="consts", bufs=1))
            persist = ctx.enter_context(tc.tile_pool(name="persist", bufs=1))

            # ---------- DMAs first, in consumption order ----------
            # (sync HWDGE drains FIFO: first X_pairs chunk to feed B0 while
            # Y lands, then phase-A inputs, then the rest)
            xpall = persist.tile([128, 8, 4, 384], f32)
            nc.sync.dma_start(
                xpall[:, 0],
                d_xp[:, ds(0, 384)].rearrange("(t p) c -> p t c", p=128),
            )
            ysb = persist.tile([128, 8, D], f32)
            nc.sync.dma_start(ysb, d_y[:, :].rearrange("(t p) d -> p t d", p=128))
            ysq = persist.tile([128, 4, D], f32)
            nc.sync.dma_start(ysq, d_yq[:, :].rearrange("(t p) d -> p t d", p=128))

            wqs = consts.tile([128, 2, D], bf16)
            nc.sync.dma_start(wqs, d_wq[:, :].rearrange("(t p) d -> p t d", p=128))
            wks = consts.tile([128, 2, D], bf16)
            nc.sync.dma_start(wks, d_wk[:, :].rearrange("(t p) d -> p t d", p=128))
            wvs = consts.tile([128, 2, D], bf16)
            nc.sync.dma_start(wvs, d_wv[:, :].rearrange("(t p) d -> p t d", p=128))
            bqs = consts.tile([1, D], bf16)
            nc.sync.dma_start(bqs, d_bq[:, :])
            bks = consts.tile([1, D], bf16)
            nc.sync.dma_start(bks, d_bk[:, :])
            bvs = consts.tile([1, D], bf16)
            nc.sync.dma_start(bvs, d_bv[:, :])
            pkcs = consts.tile([128, 8], f32)
            nc.sync.dma_start(pkcs, d_pkc[:, :])
            pqs = consts.tile([1, R], f32)
            nc.sync.dma_start(pqs, d_pqr[:, :])
            pqcs = consts.tile([1, R], f32)
            nc.sync.dma_start(pqcs, d_pqcr[:, :])

            # remaining X_pairs prefetch (6.3 MB) — overlaps phase A compute
            for kt in range(1, 8):
                nc.sync.dma_start(
                    xpall[:, kt],
                    d_xp[:, ds(384 * kt, 384)].rearrange(
                        "(t p) c -> p t c", p=128
                    ),
                )

            bdsb = consts.tile([96, H, 128], bf16)
            nc.sync.dma_start(bdsb, d_bd[:, :, :].rearrange("h p m -> p h m"))
            rbsb = consts.tile([128, H], f32)
            nc.sync.dma_start(rbsb, d_rb[:, :])
            lrsb = consts.tile([128, H, 4, 128], bf16)
            nc.sync.dma_start(lrsb, d_lr[:, :, :, :].rearrange("h c p m -> p h c m"))
            e8sb = consts.tile([8, 2, 128], bf16)
            nc.sync.dma_start(e8sb, d_e8[:, :, :])
            e8g = consts.tile([1, 64], f32)
            nc.sync.dma_start(e8g, d_eg[:, :])
            wos = consts.tile([128, 2, D], f32)
            nc.sync.dma_start(wos, d_wo[:, :].rearrange("(t p) d -> p t d", p=128))
            bos = consts.tile([1, D], f32)
            nc.sync.dma_start(bos, d_bo[:, :])

            # ---------- constants ----------
            ident = consts.tile([128, 128], f32)
            make_identity(nc, ident)
            identb = consts.tile([128, 128], bf16)
            nc.vector.tensor_copy(identb, ident)
            ones512b = consts.tile([1, 512], bf16)
            nc.vector.memset(ones512b, 1.0)
            ones128b = consts.tile([1, 128], bf16)
            nc.vector.memset(ones128b, 1.0)
            ones128f = consts.tile([1, 128], f32)
            nc.vector.memset(ones128f, 1.0)
            inv1024c = consts.tile([128, 1], bf16)
            nc.vector.memset(inv1024c, 1.0 / 1024.0)

            # ---------- persistent activations ----------
            ktsb = persist.tile([128, 2, N], bf16)     # K^T [dout, key]
            qtz = persist.tile([128, H, 512], bf16)    # per-head zero-padded Q^T
            vsb = persist.tile([128, 8, D], bf16)      # V [key, dout]
            v2sb = persist.tile([128, 8, H, 33], bf16)  # [pk*V_h | pk]
            vtsb = persist.tile([128, 2, R], f32)      # V^T of my rows
            mvt = persist.tile([128, 2], f32)          # mean_k V  (transposed col)
            xtall = persist.tile([128, 8, 4, 512], bf16)  # Xp^T (96 rows used)
            otsb = persist.tile([128, 2, R], f32)      # O^T accumulator
            pqcb = persist.tile([128, R], f32)         # (1-pq) replicated rows
            vmv = persist.tile([128, 2, R], f32)       # V^T + (1-pq)*meanV
            ohu = persist.tile([128, 2, R], f32)       # unnormalized head outs
            pq8 = persist.tile([8, R], f32)            # pq replicated to 8 rows

            nc.gpsimd.memset(qtz, 0.0)

            # ---------- phases A + B0 (interleaved) ----------
            b0pools = {}
            b0pools["xpb"] = ctx.enter_context(tc.tile_pool(name="xpb", bufs=2))

            def emit_b0(kt, pst):
                xb = b0pools["xpb"].tile([128, 4, 384], bf16, tag="xb")
                nc.gpsimd.tensor_copy(xb, xpall[:, kt])
                for cp in range(2):  # chunk pairs
                    ps = pst.tile([128, 2, 512], bf16, tag="pst")
                    for ci in range(2):
                        for qt in range(4):
                            nc.tensor.transpose(
                                ps[0:96, ci, ds(128 * qt, 128)],
                                xb[:, qt, ds(96 * (2 * cp + ci), 96)],
                                identb,
                            )
                    if cp % 2 == 0:
                        nc.scalar.copy(
                            xtall[0:96, kt, ds(2 * cp, 2)], ps[0:96]
                        )
                    else:
                        nc.vector.tensor_copy(
                            xtall[0:96, kt, ds(2 * cp, 2)], ps[0:96]
                        )

            with tc.tile_pool(name="ph_a", bufs=1) as pha, \
                 tc.tile_pool(name="ps_a", bufs=2, space="PSUM") as psa, \
                 tc.tile_pool(name="ps_t", bufs=2, space="PSUM") as pst:
                emit_b0(0, pst)
                yt = pha.tile([128, 2, N], bf16)   # Y^T full batch
                ytq = pha.tile([128, 2, R], bf16)  # Y^T my rows
                for dt_ in range(2):
                    for g in range(2):  # groups of 4 n-tiles
                        ps = psa.tile([128, 512], f32)
                        for j in range(4):
                            nt = g * 4 + j
                            nc.tensor.transpose(
                                ps[:, ds(128 * j, 128)],
                                ysb[:, nt, ds(128 * dt_, 128)],
                                ident,
                            )
                        nc.vector.tensor_copy(yt[:, dt_, ds(512 * g, 512)], ps)
                    ps = psa.tile([128, 512], f32)
                    for j in range(4):
                        nc.tensor.transpose(
                            ps[:, ds(128 * j, 128)],
                            ysq[:, j, ds(128 * dt_, 128)],
                            ident,
                        )
                    nc.vector.tensor_copy(ytq[:, dt_], ps)

                qtsb = pha.tile([128, 2, R], bf16)
                # Q^T (scaled Wq), K^T, V, V^T projections
                for dt_ in range(2):
                    ps = psa.tile([128, 512], f32)
                    for k_ in range(2):
                        nc.tensor.matmul(
                            ps, wqs[:, k_, ds(128 * dt_, 128)], ytq[:, k_],
                            start=(k_ == 0), stop=False,
                        )
                    nc.tensor.matmul(
                        ps, bqs[0:1, ds(128 * dt_, 128)], ones512b,
                        start=False, stop=True,
                    )
                    nc.vector.tensor_copy(qtsb[:, dt_], ps)

                    for half in range(2):
                        ps = psa.tile([128, 512], f32)
                        for k_ in range(2):
                            nc.tensor.matmul(
                                ps, wks[:, k_, ds(128 * dt_, 128)],
                                yt[:, k_, ds(512 * half, 512)],
                                start=(k_ == 0), stop=False,
                            )
                        nc.tensor.matmul(
                            ps, bks[0:1, ds(128 * dt_, 128)], ones512b,
                            start=False, stop=True,
                        )
                        nc.vector.tensor_copy(ktsb[:, dt_, ds(512 * half, 512)], ps)

                    ps = psa.tile([128, 512], f32)
                    for k_ in range(2):
                        nc.tensor.matmul(
                            ps, wvs[:, k_, ds(128 * dt_, 128)], ytq[:, k_],
                            start=(k_ == 0), stop=False,
                        )
                    nc.tensor.matmul(
                        ps, bvs[0:1, ds(128 * dt_, 128)], ones512b,
                        start=False, stop=True,
                    )
                    nc.vector.tensor_copy(vtsb[:, dt_], ps)

                for nt in range(8):
                    ps = psa.tile([128, 256], f32)
                    for k_ in range(2):
                        nc.tensor.matmul(
                            ps, yt[:, k_, ds(128 * nt, 128)], wvs[:, k_],
                            start=(k_ == 0), stop=False,
                        )
                    nc.tensor.matmul(ps, ones128b, bvs, start=False, stop=True)
                    nc.vector.tensor_copy(vsb[:, nt], ps)

                # per-head zero-padded Q^T slices (keeps content matmuls K=128)
                for h in range(H):
                    base = 32 * (h % 4)
                    nc.vector.tensor_copy(
                        qtz[ds(base, 32), h], qtsb[ds(base, 32), h // 4]
                    )

                # V'' = [pk * V_h | pk]
                for nt in range(8):
                    nc.vector.tensor_scalar(
                        v2sb[:, nt, :, 0:32],
                        vsb[:, nt].rearrange("p (h d) -> p h d", h=H),
                        pkcs[:, nt : nt + 1],
                        None,
                        op0=OP.mult,
                    )
                    nc.vector.tensor_copy(
                        v2sb[:, nt, :, 32:33],
                        pkcs[:, nt : nt + 1].to_broadcast((128, H, 1)),
                    )

                # mean_k V (transposed): mvt[d] = sum_n V[n, d] / 1024
                psmv = psa.tile([128, 2], f32)
                for dt_ in range(2):
                    for nt in range(8):
                        nc.tensor.matmul(
                            psmv[:, dt_ : dt_ + 1],
                            vsb[:, nt, ds(128 * dt_, 128)],
                            inv1024c,
                            start=(nt == 0), stop=(nt == 7),
                        )
                nc.vector.tensor_copy(mvt, psmv)

                # rest of B0 (X_pairs transposes)
                for kt in range(1, 8):
                    emit_b0(kt, pst)

            # ---------- phase B1: attention main loop ----------
            with tc.tile_pool(name="ps_ct", bufs=2, space="PSUM") as psct, \
                 tc.tile_pool(name="ps_z", bufs=2, space="PSUM") as psz, \
                 tc.tile_pool(name="ps_av", bufs=1, space="PSUM") as psav, \
                 tc.tile_pool(name="rz_p", bufs=4) as rzp, \
                 tc.tile_pool(name="et_p", bufs=3) as etp, \
                 tc.tile_pool(name="fin_p", bufs=2) as finp:
                # replicate (1-pq) across partitions via a K=1 outer product
                psb = psct.tile([128, 512], f32, name="psbc", tag="ct")
                nc.tensor.matmul(psb, ones128f, pqcs, start=True, stop=True)
                nc.vector.tensor_copy(pqcb, psb)
                # replicate pq to 8 rows (for the deferred normalization)
                psq = psav.tile([128, 512], f32, name="psq", tag="zg")
                nc.tensor.matmul(psq[0:8], ones128f[0:1, 0:8], pqs,
                                 start=True, stop=True)
                nc.vector.tensor_copy(pq8, psq[0:8])
                # collector for the 8 per-head softmax denominators
                z8ps = psav.tile([128, 512], f32, name="z8ps", tag="zg")
                # vmv = V^T + (1-pq)*meanV, per dim-half
                for half in range(2):
                    tmp = finp.tile([128, 512], f32, tag="fin")
                    nc.vector.tensor_scalar(
                        tmp, pqcb, mvt[:, half : half + 1], None, op0=OP.mult
                    )
                    nc.vector.tensor_add(vmv[:, half], tmp, vtsb[:, half])

                for h in range(H):
                    av = psav.tile([128, 512], f32, tag="av")
                    cts = [None] * 8
                    rzss = [None] * 8
                    ets = [None] * 8

                    def emit_lr(kt):
                        for c4 in range(4):
                            nc.tensor.matmul(
                                cts[kt], lrsb[:, h, c4],
                                rzss[kt][c4 // 2][:, c4 % 2],
                                start=False, stop=(c4 == 3),
                            )

                    def emit_exp(kt):
                        et = etp.tile([128, 512], bf16)
                        nc.scalar.activation(et, cts[kt], AF.Exp)
                        ets[kt] = et

                    def emit_av(kt):
                        nc.tensor.matmul(
                            av[0:33], v2sb[:, kt, h], ets[kt],
                            start=(kt == 0), stop=(kt == 7),
                        )

                    for kt in range(8):
                        ct = psct.tile([128, 512], f32, name="ct", tag="ct")
                        cts[kt] = ct
                        nc.tensor.matmul(
                            ct,
                            ktsb[:, h // 4, ds(128 * kt, 128)],
                            qtz[:, h],
                            start=True, stop=False,
                        )
                        rzs = []
                        for cp in range(2):
                            zps = psz.tile([128, 2, 512], f32)
                            for ci in range(2):
                                nc.tensor.matmul(
                                    zps[:, ci], bdsb[:, h],
                                    xtall[0:96, kt, 2 * cp + ci],
                                    start=True, stop=True,
                                )
                            rz = rzp.tile([128, 2, 512], bf16)
                            if cp % 2 == 0:
                                nc.scalar.activation(
                                    rz, zps, AF.Relu, bias=rbsb[:, h : h + 1]
                                )
                            else:
                                nc.vector.tensor_scalar(
                                    rz, zps, rbsb[:, h : h + 1], 0.0,
                                    op0=OP.add, op1=OP.max,
                                )
                            rzs.append(rz)
                        rzss[kt] = rzs
                        # software pipeline: the loc-reduce of tile kt-1 runs
                        # while tile kt's relu is still in flight, and the
                        # A@V of kt-2 never waits on an exp
                        if kt >= 1:
                            emit_lr(kt - 1)
                            emit_exp(kt - 1)
                        if kt >= 2:
                            emit_av(kt - 2)
                    emit_lr(7)
                    emit_av(6)
                    emit_exp(7)
                    emit_av(7)
                    # stash unnormalized result + denominator (32-aligned
                    # partition shifts only; arbitrary shifts are
                    # uncharacterized on HW)
                    nc.vector.tensor_copy(
                        ohu[ds(32 * (h % 4), 32), h // 4], av[0:32]
                    )
                    zrow = finp.tile([1, 512], f32, tag="zrow")
                    nc.vector.tensor_copy(zrow, av[32:33])
                    nc.tensor.matmul(
                        z8ps[0:8], e8g[0:1, ds(8 * h, 8)], zrow,
                        start=(h == 0), stop=(h == 7),
                    )

                # deferred normalization for all heads: 1/z = exp(-ln z)
                lnz = finp.tile([8, 512], f32, tag="fin8")
                nc.scalar.activation(lnz, z8ps[0:8], AF.Ln)
                rec8 = finp.tile([8, 512], f32, tag="fin8c")
                nc.scalar.activation(rec8, lnz, AF.Exp, scale=-1.0)
                rpq8 = finp.tile([8, 512], bf16, tag="fin8b")
                nc.vector.tensor_mul(rpq8, rec8, pq8)
                for half in range(2):
                    psb2 = psct.tile([128, 512], f32, name="psb2", tag="ct")
                    nc.tensor.matmul(
                        psb2, e8sb[:, half], rpq8, start=True, stop=True
                    )
                    t_ = finp.tile([128, 512], f32, tag="fin")
                    nc.vector.tensor_mul(t_, ohu[:, half], psb2)
                    nc.vector.tensor_add(otsb[:, half], t_, vmv[:, half])

            # ---------- phase C: O = O + relu(O @ Wo + bo) ----------
            with tc.tile_pool(name="ps_o", bufs=2, space="PSUM") as pso, \
                 tc.tile_pool(name="o_p", bufs=2) as op_:
                for j in range(4):
                    pso1 = pso.tile([128, 256], f32)
                    for dt_ in range(2):
                        nc.tensor.transpose(
                            pso1[:, ds(128 * dt_, 128)],
                            otsb[:, dt_, ds(128 * j, 128)],
                            ident,
                        )
                    oj = op_.tile([128, 256], f32)
                    nc.vector.tensor_copy(oj, pso1)

                    pso2 = pso.tile([128, 256], f32)
                    for dt_ in range(2):
                        nc.tensor.matmul(
                            pso2, otsb[:, dt_, ds(128 * j, 128)], wos[:, dt_],
                            start=(dt_ == 0), stop=False,
                        )
                    nc.tensor.matmul(pso2, ones128f, bos, start=False, stop=True)
                    r2 = op_.tile([128, 256], f32)
                    nc.scalar.activation(r2, pso2, AF.Relu)
                    ofin = op_.tile([128, 256], f32)
                    nc.vector.tensor_add(ofin, oj, r2)
                    nc.sync.dma_start(d_o[ds(128 * j, 128), :], ofin)

    _split_multiwait(nc, mybir)
    return nc


def _split_multiwait(nc, mybir):
    """This walrus build only encodes ONE sem-wait per instruction; Tile's
    tail drain carries several. Split extras onto preceding NoOps."""
    for f in nc.m.functions:
        for blk in f.blocks:
            insts = list(blk.instructions)
            changed = False
            newlist = []
            for ins in insts:
                si = ins.sync_info
                if si is not None and len(si.on_wait) > 1:
                    waits = list(si.on_wait)
                    for j, w in enumerate(waits[:-1]):
                        newlist.append(
                            mybir.InstNoOp(
                                name=f"{ins.name}_splitw{j}",
                                engine=ins.engine,
                                ins=[],
                                outs=[],
                                sync_info=mybir.SyncInfo(on_wait=[w], on_update=[]),
                            )
                        )
                    ins.sync_info = mybir.SyncInfo(
                        on_wait=[waits[-1]], on_update=list(si.on_update)
                    )
                    changed = True
                newlist.append(ins)
            if changed:
                blk.instructions = newlist


def _host_constants(Wg1, bg1, wg2, bg2):
    """Build the folded block-diag layer-1 weights, relu biases and the
    signed reduce matrices."""
    aw = np.abs(wg2)  # [H, 3]
    sw = np.sign(wg2).astype(np.float32)
    kk = np.arange(32)

    bd = np.zeros((H, 96, 128), np.float32)
    rb = np.zeros((128, H), np.float32)
    lr = np.zeros((H, 4, 128, 128), np.float32)
    for c in range(3):
        for s in range(3):
            # bd[h, 3kk+c, 4kk+s] = |wg2[h,s]| * Wg1[h,c,s]
            bd[:, 3 * kk + c, 4 * kk + s] = aw[:, s : s + 1] * Wg1[:, c, s : s + 1]
    for s in range(3):
        rb[4 * kk + s, :] = (aw[:, s] * bg1[:, s])[np.newaxis, :]
        for c4 in range(4):
            lr[:, c4, 4 * kk + s, 32 * c4 + kk] = sw[:, s : s + 1]
    # head -> output-row selector for the deferred normalization broadcast
    e8 = np.zeros((8, 2, 128), np.float32)
    for h in range(H):
        e8[h, h // 4, 32 * (h % 4) : 32 * (h % 4) + 32] = 1.0
    # per-head one-hot rows for the Z-gather matmul
    eg = np.eye(8, dtype=np.float32).reshape(1, 64)
    return bd, rb, lr, e8, eg


def kernel(**inputs):
    out, _ = _run(inputs, trace=False)
    return out


def kernel_traced(**inputs):
    return _run(inputs, trace=True)


def _run(inputs, trace=False):
    import ml_dtypes
    from concourse.bass_utils import run_bass_kernel_spmd

    bf = ml_dtypes.bfloat16

    X = {k: np.asarray(v, dtype=np.float32) for k, v in inputs.items()}
    Y = X["Y_lift"]          # [B, N, D]
    XP = X["X_pairs"]        # [B, N, N, 3]
    PQ = X["presence_q"]     # [B, N]
    PK = X["presence_k"]     # [B, N]

    bd, rb, lr, e8, eg = _host_constants(X["Wg1"], X["bg1"], X["wg2"], X["bg2"])
    bd16 = np.ascontiguousarray(bd.astype(bf))
    lr16 = np.ascontiguousarray(lr.astype(bf))
    e816 = np.ascontiguousarray(e8.astype(bf))

    wq = np.ascontiguousarray((X["Wq"] / 16.0).astype(bf))
    bq = np.ascontiguousarray((X["bq"] / 16.0).reshape(1, D).astype(bf))
    wk = np.ascontiguousarray(X["Wk"].astype(bf))
    bk = np.ascontiguousarray(X["bk"].reshape(1, D).astype(bf))
    wv = np.ascontiguousarray(X["Wv"].astype(bf))
    bv = np.ascontiguousarray(X["bv"].reshape(1, D).astype(bf))
    wo = np.ascontiguousarray(X["Wo"])
    bo = np.ascontiguousarray(X["bo"].reshape(1, D))

    if "nc" not in _CACHE:
        _CACHE["nc"] = _build_program()
    nc = _CACHE["nc"]

    in_maps = []
    for core in range(NCORES):
        b, half = core // 2, core % 2
        rows = slice(half * R, half * R + R)
        in_maps.append(
            {
                "y": np.ascontiguousarray(Y[b]),
                "yq": np.ascontiguousarray(Y[b, rows]),
                "xp": np.ascontiguousarray(XP[b, rows].reshape(R, 3 * N)),
                "pkc": np.ascontiguousarray(PK[b].reshape(8, 128).T),
                "pqr": np.ascontiguousarray(PQ[b, rows].reshape(1, R)),
                "pqcr": np.ascontiguousarray(1.0 - PQ[b, rows].reshape(1, R)),
                "wq": wq,
                "wk": wk,
                "wv": wv,
                "wo": wo,
                "bq": bq,
                "bk": bk,
                "bv": bv,
                "bo": bo,
                "bd": bd16,
                "rb": rb,
                "lr": lr16,
                "e8": e816,
                "eg": np.ascontiguousarray(eg),
            }
        )

    res = run_bass_kernel_spmd(
        nc, in_maps, core_ids=list(range(NCORES)), trace=trace
    )
    out = np.empty((B, N, D), np.float32)
    for core in range(NCORES):
        b, half = core // 2, core % 2
        out[b, half * R : half * R + R] = res.results[core]["o"]
    return out, res
